# revision 1
# baseline (speedup 1.0000x reference)
"""CrossAttnBlock TRN2 kernel: 8-way (batch x l-half) sharded, collective-free.

Reference math (b=4, c=64, h=64, w=32, dim=256, HEADS=8, l=h*w=2048):
  zf = z.reshape(b, dim, l).T            # [b, l, dim]
  q  = x.reshape(b, c, l).T              # [b, l, c]
  k  = (zf @ Wk + bk) -> [b, H, l, c];  v likewise
  S  = q @ k.T / sqrt(c); A = softmax(S, -1); P = A @ v
  out = (P heads-concat) @ Wo + bo       # [b, l, c]
  return x + out.reshape(b, c, h, w)     # raw-memory reinterpretation

Per-core (core = bi*2 + half): full K/V projection for batch bi, attention +
out-proj for l rows [half*1024, (half+1)*1024). All layouts fall out of raw
input memory: z raw = zf^T ([dim, l]), x raw = q^T ([c, l]), out rows = raw
flat output. Scores are computed transposed (S^T [m, l]) so the AV contraction
runs with m on partitions; softmax denominators come from a ones-augmented V
column; normalization is applied per-head to the [l, c]-layout out-proj
partials where the divisor is a per-partition scalar.
"""
import ml_dtypes
import numpy as np

import concourse.bass as bass
import concourse.mybir as mybir
import concourse.tile as tile
from concourse import bacc
from concourse.bass_utils import run_bass_kernel_spmd
from concourse.masks import make_identity

F32 = mybir.dt.float32
F32R = mybir.dt.float32r
BF16 = mybir.dt.bfloat16

B, C, H, W = 4, 64, 64, 32
DIM = 256
HEADS = 8
L = H * W            # 2048
LH = L // 2          # 1024 per core
INNER = HEADS * C    # 512
N_CORES = 8

_CACHE = {}


def _r(ap):
    return ap.bitcast(F32R) if ap.dtype == F32 else ap


def build_nc():
    nc = bacc.Bacc("TRN2", target_bir_lowering=False, debug=False,
                   num_devices=N_CORES)
    xq = nc.dram_tensor("xq", [C, LH], BF16, kind="ExternalInput")
    xr = nc.dram_tensor("xr", [128, LH // 128, C], F32, kind="ExternalInput")
    zb = nc.dram_tensor("zb", [DIM, L], BF16, kind="ExternalInput")
    Wk = nc.dram_tensor("Wk", [DIM, INNER], BF16, kind="ExternalInput")
    Wv = nc.dram_tensor("Wv", [DIM, INNER], BF16, kind="ExternalInput")
    Wo = nc.dram_tensor("Wo", [C, HEADS, C], BF16, kind="ExternalInput")
    bk = nc.dram_tensor("bk", [128, 4], F32, kind="ExternalInput")
    bv = nc.dram_tensor("bv", [1, INNER], BF16, kind="ExternalInput")
    bo = nc.dram_tensor("bo", [1, C], BF16, kind="ExternalInput")
    ones_b = nc.dram_tensor("ones_b", [128, 128], BF16, kind="ExternalInput")
    OUT = nc.dram_tensor("out", [LH, C], F32, kind="ExternalOutput")

    NMT = L // 128       # 16 m-tiles
    NLS = LH // 128      # 8 l-subtiles

    with tile.TileContext(nc) as tc:
        with (
            tc.tile_pool(name="const", bufs=1) as cp,
            tc.tile_pool(name="pexp", bufs=3) as pe_pool,
            tc.tile_pool(name="small", bufs=3) as sp,
            tc.tile_pool(name="ps_proj", bufs=2, space="PSUM") as ps_proj,
            tc.tile_pool(name="ps_s", bufs=2, space="PSUM") as ps_s,
            tc.tile_pool(name="ps_pt", bufs=1, space="PSUM") as ps_pt,
        ):
            # ---- constants / inputs in SBUF ----
            z_sb = [cp.tile([128, L], BF16, tag=f"z{d}", name=f"z{d}") for d in range(2)]
            for d in range(2):
                nc.sync.dma_start(out=z_sb[d], in_=zb[d * 128:(d + 1) * 128, :])
            wk_sb = [cp.tile([128, INNER], BF16, tag=f"wk{d}", name=f"wk{d}") for d in range(2)]
            wv_sb = [cp.tile([128, INNER], BF16, tag=f"wv{d}", name=f"wv{d}") for d in range(2)]
            for d in range(2):
                nc.sync.dma_start(out=wk_sb[d], in_=Wk[d * 128:(d + 1) * 128, :])
                nc.sync.dma_start(out=wv_sb[d], in_=Wv[d * 128:(d + 1) * 128, :])
            wo_sb = cp.tile([C, HEADS, C], BF16, tag="wo")
            nc.sync.dma_start(out=wo_sb, in_=Wo[:, :, :])
            x_sb = cp.tile([128, LH], BF16, tag="x")
            nc.sync.dma_start(out=x_sb[0:C, :], in_=xq[:, :])
            nc.sync.dma_start(out=x_sb[C:2 * C, :], in_=xq[:, :])
            xr_sb = cp.tile([128, NLS, C], F32, tag="xr")
            nc.sync.dma_start(out=xr_sb, in_=xr[:, :, :])
            bk_sb = cp.tile([128, 4], F32, tag="bk")
            nc.sync.dma_start(out=bk_sb, in_=bk[:, :])
            bv_sb = cp.tile([1, INNER], BF16, tag="bv")
            nc.sync.dma_start(out=bv_sb, in_=bv[:, :])
            bo_sb = cp.tile([1, C], BF16, tag="bo")
            nc.sync.dma_start(out=bo_sb, in_=bo[:, :])
            ones_bf = cp.tile([1, 128], BF16, tag="ones_bf")
            nc.sync.dma_start(out=ones_bf, in_=ones_b[0:1, :])
            ident = cp.tile([8, 8], BF16, tag="ident")
            make_identity(nc, ident)

            kT_sb = [cp.tile([128, L], BF16, tag=f"kT{t}", name=f"kT{t}") for t in range(4)]
            v_sb = cp.tile([128, NMT, HEADS, C + 1], BF16, tag="v")
            nc.sync.dma_start(
                out=v_sb[:, :, :, C:C + 1],
                in_=ones_b.rearrange("p (a b c) -> p a b c", a=NMT, b=HEADS))
            pt_sb = [cp.tile([C + 1, LH], BF16, tag=f"pt{h}", name=f"pt{h}") for h in range(HEADS)]
            sums_sb = cp.tile([HEADS, LH], BF16, tag="sums")

            # ---- Phase A: kT[ci, m] = (Wk^T @ zf^T) + bk ----
            for t in range(4):
                for s in range(4):        # m slice (512 wide)
                    pk = ps_proj.tile([128, 512], F32, tag="proj")
                    for d in range(2):
                        nc.tensor.matmul(
                            pk,
                            wk_sb[d][:, t * 128:(t + 1) * 128],
                            z_sb[d][:, s * 512:(s + 1) * 512],
                            start=(d == 0), stop=(d == 1))
                    nc.vector.tensor_scalar(
                        out=kT_sb[t][:, s * 512:(s + 1) * 512], in0=pk,
                        scalar1=bk_sb[:, t:t + 1], scalar2=None,
                        op0=mybir.AluOpType.add)
            # ---- Phase B: v[m, ci] = zf @ Wv + bv (ones col appended) ----
            for s in range(NMT):          # m tile (128 rows)
                pv = ps_proj.tile([128, 512], F32, tag="proj")
                for d in range(2):
                    nc.tensor.matmul(
                        pv,
                        z_sb[d][:, s * 128:(s + 1) * 128],
                        wv_sb[d],
                        start=(d == 0), stop=False)
                nc.tensor.matmul(pv, ones_bf, bv_sb, start=False, stop=True)
                nc.vector.tensor_copy(
                    out=v_sb[:, s, :, 0:C],
                    in_=pv.rearrange("p (h c) -> p h c", h=HEADS))

            # ---- Phase C: attention per head ----
            for h in range(HEADS):
                t, roff = h // 2, 64 * (h % 2)
                ptp = ps_pt.tile([C + 1, LH], F32, tag="pt")
                for mt in range(NMT):
                    pss = ps_s.tile([128, LH], F32, tag="s")
                    for lh_ in range(2):
                        nc.tensor.matmul(
                            pss[:, lh_ * 512:(lh_ + 1) * 512],
                            kT_sb[t][roff:roff + 64, mt * 128:(mt + 1) * 128],
                            x_sb[roff:roff + C, lh_ * 512:(lh_ + 1) * 512],
                            start=True, stop=True)
                    es = pe_pool.tile([128, LH], BF16, tag="es")
                    nc.scalar.activation(out=es, in_=pss,
                                         func=mybir.ActivationFunctionType.Exp,
                                         scale=float(C) ** -0.5)
                    for lh_ in range(2):
                        nc.tensor.matmul(
                            ptp[:, lh_ * 512:(lh_ + 1) * 512],
                            v_sb[:, mt, h, :],
                            es[:, lh_ * 512:(lh_ + 1) * 512],
                            start=(mt == 0), stop=(mt == NMT - 1))
                nc.vector.tensor_copy(out=pt_sb[h], in_=ptp)
                nc.sync.dma_start(out=sums_sb[h:h + 1, :],
                                  in_=pt_sb[h][C:C + 1, :])

            # ---- Phase D: out-proj + normalize + residual per l-subtile ----
            for ls in range(NLS):
                ptr = ps_proj.tile([128, 8], BF16, tag="proj")
                nc.tensor.transpose(ptr, sums_sb[:, ls * 128:(ls + 1) * 128], ident)
                recip = sp.tile([128, 8], F32, tag="recip")
                nc.vector.reciprocal(out=recip, in_=ptr)
                acc = None
                for h in range(HEADS):
                    po = ps_proj.tile([128, C], F32, tag="proj")
                    nc.tensor.matmul(
                        po,
                        pt_sb[h][0:C, ls * 128:(ls + 1) * 128],
                        wo_sb[:, h, :],
                        start=True, stop=(h != 0))
                    if h == 0:
                        nc.tensor.matmul(po, ones_bf, bo_sb,
                                         start=False, stop=True)
                    tmp = sp.tile([128, C], F32, tag="tmp")
                    nc.vector.tensor_scalar(
                        out=tmp, in0=po, scalar1=recip[:, h:h + 1],
                        scalar2=None, op0=mybir.AluOpType.mult)
                    if h == 0:
                        acc = sp.tile([128, C], F32, tag="oacc")
                        nc.vector.tensor_tensor(
                            out=acc, in0=xr_sb[:, ls, :], in1=tmp,
                            op=mybir.AluOpType.add)
                    else:
                        nc.vector.tensor_tensor(
                            out=acc, in0=acc, in1=tmp,
                            op=mybir.AluOpType.add)
                nc.sync.dma_start(out=OUT[ls * 128:(ls + 1) * 128, :], in_=acc)

    nc.compile()
    return nc


def kernel(x, z, Wk, bk, Wv, bv, Wo, bo):
    x = np.ascontiguousarray(x, dtype=np.float32)
    z = np.ascontiguousarray(z, dtype=np.float32)
    if "nc" not in _CACHE:
        _CACHE["nc"] = build_nc()
    nc = _CACHE["nc"]
    shared = {
        "Wk": np.ascontiguousarray(np.asarray(Wk, np.float32).astype(ml_dtypes.bfloat16)),
        "Wv": np.ascontiguousarray(np.asarray(Wv, np.float32).astype(ml_dtypes.bfloat16)),
        "Wo": np.ascontiguousarray(np.asarray(Wo, np.float32)
                                   .reshape(HEADS, C, C).transpose(1, 0, 2)
                                   .astype(ml_dtypes.bfloat16)),
        "bk": np.ascontiguousarray(
            np.asarray(bk, np.float32).reshape(4, 128).T),
        "bv": np.ascontiguousarray(
            np.asarray(bv, np.float32).reshape(1, INNER).astype(ml_dtypes.bfloat16)),
        "bo": np.ascontiguousarray(
            np.asarray(bo, np.float32).reshape(1, C).astype(ml_dtypes.bfloat16)),
        "ones_b": np.ones((128, 128), ml_dtypes.bfloat16),
    }
    in_maps = []
    for core in range(N_CORES):
        bi, half = core // 2, core % 2
        xi = x[bi].reshape(C, L)
        in_maps.append({
            "xq": np.ascontiguousarray(
                xi[:, half * LH:(half + 1) * LH].astype(ml_dtypes.bfloat16)),
            "xr": np.ascontiguousarray(
                x[bi].reshape(-1)[half * LH * C:(half + 1) * LH * C]
                .reshape(LH // 128, 128, C).transpose(1, 0, 2)),
            "zb": np.ascontiguousarray(
                z[bi].reshape(DIM, L).astype(ml_dtypes.bfloat16)),
            **shared,
        })
    _CACHE["in_maps"] = in_maps
    res = run_bass_kernel_spmd(nc, in_maps, list(range(N_CORES)))
    full = np.empty((B, L * C), dtype=np.float32)
    for core in range(N_CORES):
        bi, half = core // 2, core % 2
        full[bi, half * LH * C:(half + 1) * LH * C] = \
            res.results[core]["out"].reshape(-1)
    return full.reshape(B, C, H, W)



# revision 4
# speedup vs baseline: 1.8510x; 1.8510x over previous
"""CrossAttnBlock TRN2 kernel: 8-way (batch x l-half) sharded, collective-free.

Reference math (b=4, c=64, h=64, w=32, dim=256, HEADS=8, l=h*w=2048):
  zf = z.reshape(b, dim, l).T            # [b, l, dim]
  q  = x.reshape(b, c, l).T              # [b, l, c]
  k  = (zf @ Wk + bk) -> [b, H, l, c];  v likewise
  S  = q @ k.T / sqrt(c); A = softmax(S, -1); P = A @ v
  out = (P heads-concat) @ Wo + bo       # [b, l, c]
  return x + out.reshape(b, c, h, w)     # raw-memory reinterpretation

Exact bias simplifications (used to drop all bias matmuls on device):
  * bk adds a per-l constant over the m (softmax) axis -> softmax invariant.
  * bv adds bv to every row of P (rows of A sum to 1) -> bv @ Wo is a constant
    output offset; folded into the host-side residual tile together with bo.

Per-core (core = bi*2 + half): full K/V projection for batch bi, attention +
out-proj for l rows [half*1024, (half+1)*1024). Scores are computed transposed
(S^T [m, l], m on partitions) so the AV contraction runs with m on partitions.

Performance structure:
  * K^T is duplicated into both PE row-halves so the two m-tiles of each
    attention step run as concurrent row-tiled matmuls (contraction is c=64).
  * softmax exp is split across ScalarE (activation Exp -> fp8e4) and VectorE
    (Schraudolph exp: one fused tensor_scalar mult+add emitting fp8e4 bit
    patterns through a uint8 view).
  * A@V runs in fp8 DoubleRow mode: contraction 256 rows/instruction, with a
    ones-column in V producing the softmax denominators for free.
  * out-proj accumulates all heads into one PSUM bank; per-head 1/denominator
    scaling + accumulation is a fused scalar_tensor_tensor chain seeded with
    the host-prepared residual (x + bv@Wo + bo).
"""
import ml_dtypes
import numpy as np

import concourse.bass as bass
import concourse.mybir as mybir
import concourse.tile as tile
from concourse import bacc
from concourse.bass_utils import run_bass_kernel_spmd
from concourse.masks import make_identity

F32 = mybir.dt.float32
BF16 = mybir.dt.bfloat16
FP8 = mybir.dt.float8e4
U8 = mybir.dt.uint8

B, C, H, W = 4, 64, 64, 32
DIM = 256
HEADS = 8
L = H * W            # 2048
LH = L // 2          # 1024 per core
INNER = HEADS * C    # 512
N_CORES = 8
NLS = LH // 128      # 8 l-subtiles

SCALE = float(C) ** -0.5
EXP_A8 = 8.0 * np.log2(np.e) * SCALE   # Schraudolph slope for fp8e4m3 bits
EXP_B8 = 55.5                          # Schraudolph offset (tuned, RNE convert)
EXP_A16 = 128.0 * np.log2(np.e) * SCALE  # bf16-bits variant (fallback path)
EXP_B16 = 16255.5

USE_FP8_AV = True

_CACHE = {}


def build_nc():
    nc = bacc.Bacc("TRN2", target_bir_lowering=False, debug=False,
                   num_devices=N_CORES)
    xq = nc.dram_tensor("xq", [C, LH], BF16, kind="ExternalInput")
    xr = nc.dram_tensor("xr", [128, NLS, C], F32, kind="ExternalInput")
    zb = nc.dram_tensor("zb", [DIM, L], BF16, kind="ExternalInput")
    Wk = nc.dram_tensor("Wk", [DIM, INNER], BF16, kind="ExternalInput")
    Wv = nc.dram_tensor("Wv", [DIM, INNER], BF16, kind="ExternalInput")
    Wo = nc.dram_tensor("Wo", [C, HEADS, C], BF16, kind="ExternalInput")
    OUT = nc.dram_tensor("out", [LH, C], F32, kind="ExternalOutput")

    vdt = FP8 if USE_FP8_AV else BF16

    with tile.TileContext(nc) as tc:
        with (
            tc.tile_pool(name="const", bufs=1) as cp,
            tc.tile_pool(name="ktmp", bufs=2) as ktp,
            tc.tile_pool(name="es", bufs=4) as ep,
            tc.tile_pool(name="small", bufs=3) as sp,
            tc.tile_pool(name="ps", bufs=3, space="PSUM") as ps,
            tc.tile_pool(name="ps_pt", bufs=2, space="PSUM") as ps_pt,
        ):
            # ---- constants / inputs in SBUF ----
            z_sb = [cp.tile([128, L], BF16, tag=f"z{d}", name=f"z{d}") for d in range(2)]
            for d in range(2):
                nc.sync.dma_start(out=z_sb[d], in_=zb[d * 128:(d + 1) * 128, :])
            wk_sb = [cp.tile([128, INNER], BF16, tag=f"wk{d}", name=f"wk{d}") for d in range(2)]
            wv_sb = [cp.tile([128, INNER], BF16, tag=f"wv{d}", name=f"wv{d}") for d in range(2)]
            for d in range(2):
                nc.sync.dma_start(out=wk_sb[d], in_=Wk[d * 128:(d + 1) * 128, :])
                nc.sync.dma_start(out=wv_sb[d], in_=Wv[d * 128:(d + 1) * 128, :])
            wo_sb = cp.tile([C, HEADS, C], BF16, tag="wo")
            nc.sync.dma_start(out=wo_sb, in_=Wo[:, :, :])
            x_sb = cp.tile([128, LH], BF16, tag="x")
            nc.sync.dma_start(out=x_sb[0:C, :], in_=xq[:, :])
            nc.sync.dma_start(out=x_sb[C:2 * C, :], in_=xq[:, :])
            xr_sb = cp.tile([128, NLS, C], F32, tag="xr")
            nc.sync.dma_start(out=xr_sb, in_=xr[:, :, :])
            ident = cp.tile([8, 8], BF16, tag="ident")
            make_identity(nc, ident)

            # K^T duplicated in both PE row-halves: kT2[h][0:64]==kT2[h][64:128]
            kT2 = [cp.tile([128, L], BF16, tag=f"kT{h}", name=f"kT{h}")
                   for h in range(HEADS)]
            # V with ones column for denominators: [128, h, s, j, 80] (65 used)
            v_sb = cp.tile([128, HEADS, 8, 2, 80], vdt, tag="v")
            nc.gpsimd.memset(v_sb, 1.0)
            pt_sb = [cp.tile([C + 1, LH], BF16, tag=f"pt{h}", name=f"pt{h}")
                     for h in range(HEADS)]
            sums_sb = cp.tile([HEADS, LH], BF16, tag="sums")

            # preload the ACT exp table early (overlaps with Phase A)
            dummy = cp.tile([1, 1], F32, tag="dummy")
            nc.scalar.activation(out=dummy, in_=xr_sb[0:1, 0:1, 0],
                                 func=mybir.ActivationFunctionType.Exp)

            # ---- Phase A: kT[ci, m] = Wk^T @ zf^T, then duplicate halves ----
            for t in range(4):
                for ms in range(2):            # m-slice of 1024
                    pk = ps.tile([128, 1024], F32, tag="s")
                    for half in range(2):
                        for d in range(2):
                            nc.tensor.matmul(
                                pk[:, half * 512:(half + 1) * 512],
                                wk_sb[d][:, t * 128:(t + 1) * 128],
                                z_sb[d][:, ms * 1024 + half * 512:
                                        ms * 1024 + (half + 1) * 512],
                                start=(d == 0), stop=(d == 1))
                    kt = ktp.tile([128, 1024], BF16, tag="ktmp")
                    eng = nc.scalar if (t * 2 + ms) % 2 == 0 else nc.vector
                    if eng is nc.scalar:
                        eng.copy(out=kt, in_=pk)
                    else:
                        eng.tensor_copy(out=kt, in_=pk)
                    msl = slice(ms * 1024, (ms + 1) * 1024)
                    nc.sync.dma_start(out=kT2[2 * t][0:64, msl], in_=kt[0:64, :])
                    nc.sync.dma_start(out=kT2[2 * t][64:128, msl], in_=kt[0:64, :])
                    nc.sync.dma_start(out=kT2[2 * t + 1][0:64, msl], in_=kt[64:128, :])
                    nc.sync.dma_start(out=kT2[2 * t + 1][64:128, msl], in_=kt[64:128, :])

            # ---- Phase B: v[m, (h c)] = zf @ Wv -> fp8 per-head tiles ----
            for mt in range(16):
                s, j = mt // 2, mt % 2
                pv = ps.tile([128, 1024], F32, tag="s")
                for d in range(2):
                    nc.tensor.matmul(
                        pv[:, 0:512],
                        z_sb[d][:, mt * 128:(mt + 1) * 128],
                        wv_sb[d],
                        start=(d == 0), stop=(d == 1))
                src = pv[:, 0:512].rearrange("p (h c) -> p h c", h=HEADS)
                dst = v_sb[:, :, s, j, 0:C]
                if mt % 2 == 0:
                    nc.scalar.copy(out=dst, in_=src)
                else:
                    nc.vector.tensor_copy(out=dst, in_=src)

            # ---- Phase C: attention ----
            for lh in range(2):
                lsl = slice(lh * 512, (lh + 1) * 512)
                for h in range(HEADS):
                    ptp = ps_pt.tile([C + 1, 512], F32, tag="pt")
                    for s in range(8):
                        pss = ps.tile([128, 1024], F32, tag="s")
                        for j in range(2):     # row-tiled concurrent pair
                            mt = 2 * s + j
                            nc.tensor.matmul(
                                pss[:, j * 512:(j + 1) * 512],
                                kT2[h][64 * j:64 * j + 64,
                                       mt * 128:(mt + 1) * 128],
                                x_sb[64 * j:64 * j + C, lsl],
                                start=True, stop=True)
                        es = ep.tile([128, 2, 512], vdt, tag="es")
                        if (s + h) % 2 == 0:
                            nc.scalar.activation(
                                out=es, in_=pss.rearrange("p (a b) -> p a b", a=2),
                                func=mybir.ActivationFunctionType.Exp,
                                scale=SCALE)
                        else:
                            if USE_FP8_AV:
                                nc.vector.tensor_scalar(
                                    out=es.bitcast(U8),
                                    in0=pss.rearrange("p (a b) -> p a b", a=2),
                                    scalar1=EXP_A8, scalar2=EXP_B8,
                                    op0=mybir.AluOpType.mult,
                                    op1=mybir.AluOpType.add)
                            else:
                                nc.vector.tensor_scalar(
                                    out=es.bitcast(mybir.dt.int16),
                                    in0=pss.rearrange("p (a b) -> p a b", a=2),
                                    scalar1=EXP_A16, scalar2=EXP_B16,
                                    op0=mybir.AluOpType.mult,
                                    op1=mybir.AluOpType.add)
                        if USE_FP8_AV:
                            nc.tensor.matmul(
                                ptp, v_sb[:, h, s, :, 0:C + 1], es,
                                start=(s == 0), stop=(s == 7),
                                perf_mode=mybir.MatmulPerfMode.DoubleRow)
                        else:
                            for j in range(2):
                                nc.tensor.matmul(
                                    ptp, v_sb[:, h, s, j, 0:C + 1], es[:, j, :],
                                    start=(s == 0 and j == 0),
                                    stop=(s == 7 and j == 1))
                    if h % 2 == 0:
                        nc.scalar.copy(out=pt_sb[h][:, lsl], in_=ptp)
                    else:
                        nc.vector.tensor_copy(out=pt_sb[h][:, lsl], in_=ptp)
                    nc.sync.dma_start(out=sums_sb[h:h + 1, lsl],
                                      in_=pt_sb[h][C:C + 1, lsl])

            # ---- Phase D: out-proj + normalize + residual per l-subtile ----
            for ls in range(NLS):
                lblk = slice(ls * 128, (ls + 1) * 128)
                ptr = ps.tile([128, 1024], F32, tag="s")
                ptr_b = ptr[:, 0:4].bitcast(BF16)
                nc.tensor.transpose(ptr_b, sums_sb[:, lblk], ident)
                recip = sp.tile([128, 8], F32, tag="recip")
                nc.vector.reciprocal(out=recip, in_=ptr_b)
                acc = sp.tile([128, C], F32, tag="oacc")
                for h in range(HEADS):
                    po = ps.tile([128, 1024], F32, tag="s")
                    nc.tensor.matmul(
                        po[:, 0:C],
                        pt_sb[h][0:C, lblk],
                        wo_sb[:, h, :],
                        start=True, stop=True)
                    nc.vector.scalar_tensor_tensor(
                        out=acc,
                        in0=po[:, 0:C],
                        scalar=recip[:, h:h + 1],
                        in1=xr_sb[:, ls, :] if h == 0 else acc,
                        op0=mybir.AluOpType.mult,
                        op1=mybir.AluOpType.add)
                nc.sync.dma_start(out=OUT[lblk, :], in_=acc)

    nc.compile()
    return nc


def kernel(x, z, Wk, bk, Wv, bv, Wo, bo):
    x = np.ascontiguousarray(x, dtype=np.float32)
    z = np.ascontiguousarray(z, dtype=np.float32)
    if "nc" not in _CACHE:
        _CACHE["nc"] = build_nc()
    nc = _CACHE["nc"]
    # bv/bo fold: P rows are convex-combination outputs plus bv, so the output
    # picks up the constant bv @ Wo + bo; bk is softmax-shift-invariant.
    bo_eff = (np.asarray(bv, np.float32) @ np.asarray(Wo, np.float32)
              + np.asarray(bo, np.float32))
    shared = {
        "Wk": np.ascontiguousarray(np.asarray(Wk, np.float32).astype(ml_dtypes.bfloat16)),
        "Wv": np.ascontiguousarray(np.asarray(Wv, np.float32).astype(ml_dtypes.bfloat16)),
        "Wo": np.ascontiguousarray(np.asarray(Wo, np.float32)
                                   .reshape(HEADS, C, C).transpose(1, 0, 2)
                                   .astype(ml_dtypes.bfloat16)),
    }
    in_maps = []
    for core in range(N_CORES):
        bi, half = core // 2, core % 2
        xi = x[bi].reshape(C, L)
        xr = (x[bi].reshape(-1)[half * LH * C:(half + 1) * LH * C]
              .reshape(LH // 128, 128, C).transpose(1, 0, 2)
              + bo_eff[None, None, :])
        in_maps.append({
            "xq": np.ascontiguousarray(
                xi[:, half * LH:(half + 1) * LH].astype(ml_dtypes.bfloat16)),
            "xr": np.ascontiguousarray(xr),
            "zb": np.ascontiguousarray(
                z[bi].reshape(DIM, L).astype(ml_dtypes.bfloat16)),
            **shared,
        })
    _CACHE["in_maps"] = in_maps
    res = run_bass_kernel_spmd(nc, in_maps, list(range(N_CORES)))
    full = np.empty((B, L * C), dtype=np.float32)
    for core in range(N_CORES):
        bi, half = core // 2, core % 2
        full[bi, half * LH * C:(half + 1) * LH * C] = \
            res.results[core]["out"].reshape(-1)
    return full.reshape(B, C, H, W)


# revision 8
# speedup vs baseline: 2.0400x; 1.1021x over previous
"""CrossAttnBlock TRN2 kernel: 8-way (batch x l-half) sharded, collective-free.

Reference math (b=4, c=64, h=64, w=32, dim=256, HEADS=8, l=h*w=2048):
  zf = z.reshape(b, dim, l).T            # [b, l, dim]
  q  = x.reshape(b, c, l).T              # [b, l, c]
  k  = (zf @ Wk + bk) -> [b, H, l, c];  v likewise
  S  = q @ k.T / sqrt(c); A = softmax(S, -1); P = A @ v
  out = (P heads-concat) @ Wo + bo       # [b, l, c]
  return x + out.reshape(b, c, h, w)     # raw-memory reinterpretation

Exact bias simplifications (used to drop all bias matmuls on device):
  * bk adds a per-l constant over the m (softmax) axis -> softmax invariant.
  * bv adds bv to every row of P (rows of A sum to 1) -> bv @ Wo is a constant
    output offset; folded into the host-side residual tile together with bo.

Per-core (core = bi*2 + half): full K/V projection for batch bi, attention +
out-proj for l rows [half*1024, (half+1)*1024). Scores are computed transposed
(S^T [m, l], m on partitions) so the AV contraction runs with m on partitions.

Performance structure:
  * K^T is duplicated into both PE row-halves so the two m-tiles of each
    attention step run as concurrent row-tiled matmuls (contraction is c=64).
  * softmax exp is split across ScalarE (activation Exp -> fp8e4) and VectorE
    (Schraudolph exp: one fused tensor_scalar mult+add emitting fp8e4 bit
    patterns through a uint8 view).
  * A@V runs in fp8 DoubleRow mode: contraction 256 rows/instruction, with a
    ones-column in V producing the softmax denominators for free.
  * out-proj accumulates all heads into one PSUM bank; per-head 1/denominator
    scaling + accumulation is a fused scalar_tensor_tensor chain seeded with
    the host-prepared residual (x + bv@Wo + bo).
"""
import ml_dtypes
import numpy as np

import concourse.bass as bass
import concourse.mybir as mybir
import concourse.tile as tile
from concourse import bacc
from concourse.bass_utils import run_bass_kernel_spmd
from concourse.masks import make_identity

F32 = mybir.dt.float32
BF16 = mybir.dt.bfloat16
FP8 = mybir.dt.float8e4
U8 = mybir.dt.uint8

B, C, H, W = 4, 64, 64, 32
DIM = 256
HEADS = 8
L = H * W            # 2048
LH = L // 2          # 1024 per core
INNER = HEADS * C    # 512
N_CORES = 8
NLS = LH // 128      # 8 l-subtiles

SCALE = float(C) ** -0.5
EXP_A8 = 8.0 * np.log2(np.e) * SCALE   # Schraudolph slope for fp8e4m3 bits
EXP_B8 = 55.5                          # Schraudolph offset (tuned, RNE convert)
EXP_A16 = 128.0 * np.log2(np.e) * SCALE  # bf16-bits variant (fallback path)
EXP_B16 = 16255.5

USE_FP8_AV = True

_CACHE = {}


def build_nc():
    nc = bacc.Bacc("TRN2", target_bir_lowering=False, debug=False,
                   num_devices=N_CORES)
    xq = nc.dram_tensor("xq", [C, LH], BF16, kind="ExternalInput")
    xr = nc.dram_tensor("xr", [128, NLS, C], F32, kind="ExternalInput")
    zb = nc.dram_tensor("zb", [DIM, L], BF16, kind="ExternalInput")
    Wk = nc.dram_tensor("Wk", [DIM, INNER], BF16, kind="ExternalInput")
    Wv = nc.dram_tensor("Wv", [DIM, INNER], BF16, kind="ExternalInput")
    Wo = nc.dram_tensor("Wo", [C, HEADS, C], BF16, kind="ExternalInput")
    OUT = nc.dram_tensor("out", [LH, C], F32, kind="ExternalOutput")

    vdt = FP8 if USE_FP8_AV else BF16

    with tile.TileContext(nc) as tc:
        with (
            tc.tile_pool(name="const", bufs=1) as cp,
            tc.tile_pool(name="ktmp", bufs=2) as ktp,
            tc.tile_pool(name="es", bufs=4) as ep,
            tc.tile_pool(name="small", bufs=3) as sp,
            tc.tile_pool(name="ps", bufs=3, space="PSUM") as ps,
            tc.tile_pool(name="ps_pt", bufs=2, space="PSUM") as ps_pt,
        ):
            # ---- constants / inputs in SBUF ----
            z_sb = [cp.tile([128, L], BF16, tag=f"z{d}", name=f"z{d}") for d in range(2)]
            for d in range(2):
                nc.sync.dma_start(out=z_sb[d], in_=zb[d * 128:(d + 1) * 128, :])
            wk_sb = [cp.tile([128, INNER], BF16, tag=f"wk{d}", name=f"wk{d}") for d in range(2)]
            wv_sb = [cp.tile([128, INNER], BF16, tag=f"wv{d}", name=f"wv{d}") for d in range(2)]
            for d in range(2):
                nc.sync.dma_start(out=wk_sb[d], in_=Wk[d * 128:(d + 1) * 128, :])
                nc.sync.dma_start(out=wv_sb[d], in_=Wv[d * 128:(d + 1) * 128, :])
            wo_sb = cp.tile([C, HEADS, C], BF16, tag="wo")
            nc.sync.dma_start(out=wo_sb, in_=Wo[:, :, :])
            x_sb = cp.tile([128, LH], BF16, tag="x")
            nc.sync.dma_start(out=x_sb[0:C, :], in_=xq[:, :])
            nc.sync.dma_start(out=x_sb[C:2 * C, :], in_=xq[:, :])
            xr_sb = cp.tile([128, NLS, C], F32, tag="xr")
            nc.sync.dma_start(out=xr_sb, in_=xr[:, :, :])
            ident = cp.tile([8, 8], BF16, tag="ident")
            make_identity(nc, ident)

            # K^T duplicated in both PE row-halves: kT2[h][0:64]==kT2[h][64:128]
            kT2 = [cp.tile([128, L], BF16, tag=f"kT{h}", name=f"kT{h}")
                   for h in range(HEADS)]
            # V with ones column for denominators: [128, h, s, j, 80] (65 used)
            v_sb = cp.tile([128, HEADS, 8, 2, 80], vdt, tag="v")
            nc.gpsimd.memset(v_sb, 1.0)
            pt_sb = [cp.tile([C + 1, LH], BF16, tag=f"pt{h}", name=f"pt{h}")
                     for h in range(HEADS)]
            sums_sb = cp.tile([HEADS, LH], BF16, tag="sums")

            # preload the ACT exp table early (overlaps with Phase A)
            dummy = cp.tile([1, 1], F32, tag="dummy")
            nc.scalar.activation(out=dummy, in_=xr_sb[0:1, 0:1, 0],
                                 func=mybir.ActivationFunctionType.Exp)

            # ---- Phase A: kT2[h][(dup), m] = Wk_h^T @ zf^T, both row-halves.
            # The lhsT repeats head h's 64 weight columns twice (stride-0 dim)
            # so the matmul writes kT_h into partitions 0:64 AND 64:128.
            for h in range(HEADS):
                for ms in range(2):            # m-slice of 1024
                    pk = ps.tile([128, 1024], F32, tag="s")
                    for half in range(2):
                        csl = slice(ms * 1024 + half * 512,
                                    ms * 1024 + (half + 1) * 512)
                        psl = slice(half * 512, (half + 1) * 512)
                        for rep in range(2):   # col-tiled pair: both row-halves
                            for d in range(2):
                                nc.tensor.matmul(
                                    pk[rep * C:(rep + 1) * C, psl],
                                    wk_sb[d][:, h * C:(h + 1) * C],
                                    z_sb[d][:, csl],
                                    start=(d == 0), stop=(d == 1))
                    msl = slice(ms * 1024, (ms + 1) * 1024)
                    if (h * 2 + ms) % 2 == 0:
                        nc.scalar.copy(out=kT2[h][:, msl], in_=pk)
                    else:
                        nc.vector.tensor_copy(out=kT2[h][:, msl], in_=pk)

            # ---- Phase B: v[m, (h c)] = zf @ Wv -> fp8 per-head tiles ----
            for mt in range(16):
                s, j = mt // 2, mt % 2
                pv = ps.tile([128, 1024], F32, tag="s")
                for d in range(2):
                    nc.tensor.matmul(
                        pv[:, 0:512],
                        z_sb[d][:, mt * 128:(mt + 1) * 128],
                        wv_sb[d],
                        start=(d == 0), stop=(d == 1))
                src = pv[:, 0:512].rearrange("p (h c) -> p h c", h=HEADS)
                dst = v_sb[:, :, s, j, 0:C]
                if mt % 2 == 0:
                    nc.scalar.copy(out=dst, in_=src)
                else:
                    nc.vector.tensor_copy(out=dst, in_=src)

            # ---- Phase C: attention ----
            def emit_qk(h, s, lh):
                lsl = slice(lh * 512, (lh + 1) * 512)
                pss = ps.tile([128, 1024], F32, tag="s")
                for j in range(2):             # row-tiled concurrent pair
                    mt = 2 * s + j
                    nc.tensor.matmul(
                        pss[:, j * 512:(j + 1) * 512],
                        kT2[h][64 * j:64 * j + 64, mt * 128:(mt + 1) * 128],
                        x_sb[64 * j:64 * j + C, lsl],
                        start=True, stop=True)
                es = ep.tile([128, 2, 512], vdt, tag="es")
                if ((h * 8 + s) * 15) % 32 < 15:   # ~60/128 tiles on ScalarE
                    nc.scalar.activation(
                        out=es, in_=pss.rearrange("p (a b) -> p a b", a=2),
                        func=mybir.ActivationFunctionType.Exp,
                        scale=SCALE)
                else:
                    nc.vector.tensor_scalar(
                        out=es.bitcast(U8),
                        in0=pss.rearrange("p (a b) -> p a b", a=2),
                        scalar1=EXP_A8, scalar2=EXP_B8,
                        op0=mybir.AluOpType.mult,
                        op1=mybir.AluOpType.add)
                return es

            def emit_av(h, s, es, ptp):
                nc.tensor.matmul(
                    ptp, v_sb[:, h, s, :, 0:C + 1], es,
                    start=(s == 0), stop=(s == 7),
                    perf_mode=mybir.MatmulPerfMode.DoubleRow)

            def emit_head(h, lh):
                # AV issue lags QK by 2 so the PE queue head never waits
                lsl = slice(lh * 512, (lh + 1) * 512)
                ptp = ps_pt.tile([C + 1, 512], F32, tag="pt")
                pend = []
                for s in range(8):
                    pend.append((s, emit_qk(h, s, lh)))
                    if len(pend) > 2:
                        ss, ees = pend.pop(0)
                        emit_av(h, ss, ees, ptp)
                for ss, ees in pend:
                    emit_av(h, ss, ees, ptp)
                nc.scalar.copy(out=pt_sb[h][:, lsl], in_=ptp)
                nc.sync.dma_start(out=sums_sb[h:h + 1, lsl],
                                  in_=pt_sb[h][C:C + 1, lsl])

            # ---- Phase D emitters ----
            def emit_d(ls):
                lblk = slice(ls * 128, (ls + 1) * 128)
                ptr = ps.tile([128, 1024], F32, tag="s")
                ptr_b = ptr[:, 0:4].bitcast(BF16)
                nc.tensor.transpose(ptr_b, sums_sb[:, lblk], ident)
                recip = sp.tile([128, 8], F32, tag="recip")
                nc.vector.reciprocal(out=recip, in_=ptr_b)
                po = ps.tile([128, 1024], F32, tag="s")
                for h in range(HEADS):
                    nc.tensor.matmul(
                        po[:, h * C:(h + 1) * C],
                        pt_sb[h][0:C, lblk],
                        wo_sb[:, h, :],
                        start=(h == 0), stop=(h == 7))
                # heads 4-7: ScalarE scale-copies; heads 0-3: fused STT on DVE
                tmp = sp.tile([128, 4, C], F32, tag="dtmp")
                for i in range(4):
                    nc.scalar.mul(out=tmp[:, i, :], in_=po[:, (4 + i) * C:(5 + i) * C],
                                  mul=recip[:, 4 + i:5 + i])
                acc = sp.tile([128, C], F32, tag="oacc")
                for h in range(4):
                    nc.vector.scalar_tensor_tensor(
                        out=acc, in0=po[:, h * C:(h + 1) * C],
                        scalar=recip[:, h:h + 1],
                        in1=xr_sb[:, ls, :] if h == 0 else acc,
                        op0=mybir.AluOpType.mult, op1=mybir.AluOpType.add)
                for i in range(4):
                    nc.vector.tensor_tensor(
                        out=acc, in0=acc, in1=tmp[:, i, :],
                        op=mybir.AluOpType.add)
                nc.sync.dma_start(out=OUT[lblk, :], in_=acc)

            for h in range(HEADS):
                emit_head(h, 0)
            for ls in range(4):
                emit_d(ls)
            for h in range(HEADS):
                emit_head(h, 1)
            for ls in range(4, 8):
                emit_d(ls)

    nc.compile()
    return nc


def kernel(x, z, Wk, bk, Wv, bv, Wo, bo):
    x = np.ascontiguousarray(x, dtype=np.float32)
    z = np.ascontiguousarray(z, dtype=np.float32)
    if "nc" not in _CACHE:
        _CACHE["nc"] = build_nc()
    nc = _CACHE["nc"]
    # bv/bo fold: P rows are convex-combination outputs plus bv, so the output
    # picks up the constant bv @ Wo + bo; bk is softmax-shift-invariant.
    bo_eff = (np.asarray(bv, np.float32) @ np.asarray(Wo, np.float32)
              + np.asarray(bo, np.float32))
    shared = {
        "Wk": np.ascontiguousarray(np.asarray(Wk, np.float32).astype(ml_dtypes.bfloat16)),
        "Wv": np.ascontiguousarray(np.asarray(Wv, np.float32).astype(ml_dtypes.bfloat16)),
        "Wo": np.ascontiguousarray(np.asarray(Wo, np.float32)
                                   .reshape(HEADS, C, C).transpose(1, 0, 2)
                                   .astype(ml_dtypes.bfloat16)),
    }
    in_maps = []
    for core in range(N_CORES):
        bi, half = core // 2, core % 2
        xi = x[bi].reshape(C, L)
        xr = (x[bi].reshape(-1)[half * LH * C:(half + 1) * LH * C]
              .reshape(LH // 128, 128, C).transpose(1, 0, 2)
              + bo_eff[None, None, :])
        in_maps.append({
            "xq": np.ascontiguousarray(
                xi[:, half * LH:(half + 1) * LH].astype(ml_dtypes.bfloat16)),
            "xr": np.ascontiguousarray(xr),
            "zb": np.ascontiguousarray(
                z[bi].reshape(DIM, L).astype(ml_dtypes.bfloat16)),
            **shared,
        })
    _CACHE["in_maps"] = in_maps
    res = run_bass_kernel_spmd(nc, in_maps, list(range(N_CORES)))
    full = np.empty((B, L * C), dtype=np.float32)
    for core in range(N_CORES):
        bi, half = core // 2, core % 2
        full[bi, half * LH * C:(half + 1) * LH * C] = \
            res.results[core]["out"].reshape(-1)
    return full.reshape(B, C, H, W)


# revision 11
# speedup vs baseline: 2.1487x; 1.0533x over previous
"""CrossAttnBlock TRN2 kernel: 8-way (batch x l-half) sharded, collective-free.

Reference math (b=4, c=64, h=64, w=32, dim=256, HEADS=8, l=h*w=2048):
  zf = z.reshape(b, dim, l).T            # [b, l, dim]
  q  = x.reshape(b, c, l).T              # [b, l, c]
  k  = (zf @ Wk + bk) -> [b, H, l, c];  v likewise
  S  = q @ k.T / sqrt(c); A = softmax(S, -1); P = A @ v
  out = (P heads-concat) @ Wo + bo       # [b, l, c]
  return x + out.reshape(b, c, h, w)     # raw-memory reinterpretation

Exact bias simplifications (used to drop all bias matmuls on device):
  * bk adds a per-l constant over the m (softmax) axis -> softmax invariant.
  * bv adds bv to every row of P (rows of A sum to 1) -> bv @ Wo is a constant
    output offset; folded into the host-side residual tile together with bo.

Per-core (core = bi*2 + half): full K/V projection for batch bi, attention +
out-proj for l rows [half*1024, (half+1)*1024). Scores are computed transposed
(S^T [m, l], m on partitions) so the AV contraction runs with m on partitions.

Performance structure:
  * K^T is duplicated into both PE row-halves so the two m-tiles of each
    attention step run as concurrent row-tiled matmuls (contraction is c=64).
  * softmax exp is split across ScalarE (activation Exp -> fp8e4) and VectorE
    (Schraudolph exp: one fused tensor_scalar mult+add emitting fp8e4 bit
    patterns through a uint8 view).
  * A@V runs in fp8 DoubleRow mode: contraction 256 rows/instruction, with a
    ones-column in V producing the softmax denominators for free.
  * out-proj accumulates all heads into one PSUM bank; per-head 1/denominator
    scaling + accumulation is a fused scalar_tensor_tensor chain seeded with
    the host-prepared residual (x + bv@Wo + bo).
"""
import ml_dtypes
import numpy as np

import concourse.bass as bass
import concourse.mybir as mybir
import concourse.tile as tile
from concourse import bacc
from concourse.bass_utils import run_bass_kernel_spmd
from concourse.masks import make_identity

F32 = mybir.dt.float32
BF16 = mybir.dt.bfloat16
FP8 = mybir.dt.float8e4
U8 = mybir.dt.uint8

B, C, H, W = 4, 64, 64, 32
DIM = 256
HEADS = 8
L = H * W            # 2048
LH = L // 2          # 1024 per core
INNER = HEADS * C    # 512
N_CORES = 8
NLS = LH // 128      # 8 l-subtiles

SCALE = float(C) ** -0.5
EXP_A8 = 8.0 * np.log2(np.e) * SCALE   # Schraudolph slope for fp8e4m3 bits
EXP_B8 = 55.5                          # Schraudolph offset (tuned, RNE convert)
EXP_A16 = 128.0 * np.log2(np.e) * SCALE  # bf16-bits variant (fallback path)
EXP_B16 = 16255.5

USE_FP8_AV = True

_CACHE = {}


def build_nc():
    nc = bacc.Bacc("TRN2", target_bir_lowering=False, debug=False,
                   num_devices=N_CORES)
    xq = nc.dram_tensor("xq", [C, LH], BF16, kind="ExternalInput")
    xr = nc.dram_tensor("xr", [128, NLS, C], F32, kind="ExternalInput")
    zb = nc.dram_tensor("zb", [DIM, L], BF16, kind="ExternalInput")
    Wk = nc.dram_tensor("Wk", [DIM, INNER], BF16, kind="ExternalInput")
    Wv = nc.dram_tensor("Wv", [DIM, INNER], BF16, kind="ExternalInput")
    Wo = nc.dram_tensor("Wo", [C, HEADS, C], BF16, kind="ExternalInput")
    OUT = nc.dram_tensor("out", [LH, C], F32, kind="ExternalOutput")

    vdt = FP8 if USE_FP8_AV else BF16

    with tile.TileContext(nc) as tc:
        with (
            tc.tile_pool(name="const", bufs=1) as cp,
            tc.tile_pool(name="ktmp", bufs=2) as ktp,
            tc.tile_pool(name="es", bufs=6) as ep,
            tc.tile_pool(name="small", bufs=3) as sp,
            tc.tile_pool(name="ps", bufs=3, space="PSUM") as ps,
            tc.tile_pool(name="ps_pt", bufs=2, space="PSUM") as ps_pt,
        ):
            # ---- constants / inputs in SBUF ----
            z_sb = [cp.tile([128, L], BF16, tag=f"z{d}", name=f"z{d}") for d in range(2)]
            for d in range(2):
                nc.sync.dma_start(out=z_sb[d], in_=zb[d * 128:(d + 1) * 128, :])
            wk_sb = [cp.tile([128, INNER], BF16, tag=f"wk{d}", name=f"wk{d}") for d in range(2)]
            wv_sb = [cp.tile([128, INNER], BF16, tag=f"wv{d}", name=f"wv{d}") for d in range(2)]
            for d in range(2):
                nc.sync.dma_start(out=wk_sb[d], in_=Wk[d * 128:(d + 1) * 128, :])
                nc.sync.dma_start(out=wv_sb[d], in_=Wv[d * 128:(d + 1) * 128, :])
            wo_sb = cp.tile([C, HEADS, C], BF16, tag="wo")
            nc.sync.dma_start(out=wo_sb, in_=Wo[:, :, :])
            x_sb = cp.tile([128, LH], BF16, tag="x")
            nc.sync.dma_start(out=x_sb[0:C, :], in_=xq[:, :])
            nc.sync.dma_start(out=x_sb[C:2 * C, :], in_=xq[:, :])
            xr_sb = cp.tile([128, NLS, C], F32, tag="xr")
            nc.sync.dma_start(out=xr_sb, in_=xr[:, :, :])
            ident = cp.tile([8, 8], BF16, tag="ident")
            make_identity(nc, ident)

            # K^T duplicated in both PE row-halves: kT2[h][0:64]==kT2[h][64:128]
            kT2 = [cp.tile([128, L], BF16, tag=f"kT{h}", name=f"kT{h}")
                   for h in range(HEADS)]
            # V with ones column for denominators: [128, h, s, j, 80] (65 used)
            v_sb = cp.tile([128, HEADS, 8, 2, 80], vdt, tag="v")
            nc.gpsimd.memset(v_sb, 1.0)
            pt_sb = [cp.tile([C + 1, LH], BF16, tag=f"pt{h}", name=f"pt{h}")
                     for h in range(HEADS)]
            sums_sb = cp.tile([HEADS, LH], BF16, tag="sums")

            # preload the ACT exp table early (overlaps with Phase A)
            dummy = cp.tile([1, 1], F32, tag="dummy")
            nc.scalar.activation(out=dummy, in_=xr_sb[0:1, 0:1, 0],
                                 func=mybir.ActivationFunctionType.Exp)

            # ---- Phase A: kT2[h][(dup), m] = Wk_h^T @ zf^T, both row-halves.
            # The lhsT repeats head h's 64 weight columns twice (stride-0 dim)
            # so the matmul writes kT_h into partitions 0:64 AND 64:128.
            for h in range(HEADS):
                for ms in range(2):            # m-slice of 1024
                    pk = ps.tile([128, 1024], F32, tag="s")
                    for half in range(2):
                        csl = slice(ms * 1024 + half * 512,
                                    ms * 1024 + (half + 1) * 512)
                        psl = slice(half * 512, (half + 1) * 512)
                        for rep in range(2):   # col-tiled pair: both row-halves
                            for d in range(2):
                                nc.tensor.matmul(
                                    pk[rep * C:(rep + 1) * C, psl],
                                    wk_sb[d][:, h * C:(h + 1) * C],
                                    z_sb[d][:, csl],
                                    start=(d == 0), stop=(d == 1))
                    msl = slice(ms * 1024, (ms + 1) * 1024)
                    if (h * 2 + ms) % 2 == 0:
                        nc.scalar.copy(out=kT2[h][:, msl], in_=pk)
                    else:
                        nc.vector.tensor_copy(out=kT2[h][:, msl], in_=pk)

            # ---- Phase B: v[m, (h c)] = zf @ Wv -> fp8 per-head tiles ----
            for mt in range(16):
                s, j = mt // 2, mt % 2
                pv = ps.tile([128, 1024], F32, tag="s")
                for d in range(2):
                    nc.tensor.matmul(
                        pv[:, 0:512],
                        z_sb[d][:, mt * 128:(mt + 1) * 128],
                        wv_sb[d],
                        start=(d == 0), stop=(d == 1))
                src = pv[:, 0:512].rearrange("p (h c) -> p h c", h=HEADS)
                dst = v_sb[:, :, s, j, 0:C]
                if mt % 2 == 0:
                    nc.scalar.copy(out=dst, in_=src)
                else:
                    nc.vector.tensor_copy(out=dst, in_=src)

            # ---- Phase C: attention ----
            def emit_qk(h, s, lh):
                lsl = slice(lh * 512, (lh + 1) * 512)
                pss = ps.tile([128, 1024], F32, tag="s")
                for j in range(2):             # row-tiled concurrent pair
                    mt = 2 * s + j
                    nc.tensor.matmul(
                        pss[:, j * 512:(j + 1) * 512],
                        kT2[h][64 * j:64 * j + 64, mt * 128:(mt + 1) * 128],
                        x_sb[64 * j:64 * j + C, lsl],
                        start=True, stop=True)
                es = ep.tile([128, 2, 512], vdt, tag="es")
                if ((h * 8 + s) * 15) % 32 < 15:   # ~60/128 tiles on ScalarE
                    nc.scalar.activation(
                        out=es, in_=pss.rearrange("p (a b) -> p a b", a=2),
                        func=mybir.ActivationFunctionType.Exp,
                        scale=SCALE)
                else:
                    nc.vector.tensor_scalar(
                        out=es.bitcast(U8),
                        in0=pss.rearrange("p (a b) -> p a b", a=2),
                        scalar1=EXP_A8, scalar2=EXP_B8,
                        op0=mybir.AluOpType.mult,
                        op1=mybir.AluOpType.add)
                return es

            def emit_av(h, s, es, ptp):
                nc.tensor.matmul(
                    ptp, v_sb[:, h, s, :, 0:C + 1], es,
                    start=(s == 0), stop=(s == 7),
                    perf_mode=mybir.MatmulPerfMode.DoubleRow)

            def emit_head(h, lh):
                # AV issue lags QK by 2 so the PE queue head never waits
                lsl = slice(lh * 512, (lh + 1) * 512)
                ptp = ps_pt.tile([C + 1, 512], F32, tag="pt")
                pend = []
                for s in range(8):
                    pend.append((s, emit_qk(h, s, lh)))
                    if len(pend) > 3:
                        ss, ees = pend.pop(0)
                        emit_av(h, ss, ees, ptp)
                for ss, ees in pend:
                    emit_av(h, ss, ees, ptp)
                nc.scalar.copy(out=pt_sb[h][:, lsl], in_=ptp)
                nc.sync.dma_start(out=sums_sb[h:h + 1, lsl],
                                  in_=pt_sb[h][C:C + 1, lsl])

            # ---- Phase D emitters ----
            def emit_d(ls):
                lblk = slice(ls * 128, (ls + 1) * 128)
                ptr = ps.tile([128, 1024], F32, tag="s")
                ptr_b = ptr[:, 0:4].bitcast(BF16)
                nc.tensor.transpose(ptr_b, sums_sb[:, lblk], ident)
                recip = sp.tile([128, 8], F32, tag="recip")
                nc.vector.reciprocal(out=recip, in_=ptr_b)
                po = ps.tile([128, 1024], F32, tag="s")
                for h in range(HEADS):
                    nc.tensor.matmul(
                        po[:, h * C:(h + 1) * C],
                        pt_sb[h][0:C, lblk],
                        wo_sb[:, h, :],
                        start=(h == 0), stop=(h == 7))
                # heads 4-7: ScalarE scale-copies; heads 0-3: fused STT on DVE
                tmp = sp.tile([128, 4, C], F32, tag="dtmp")
                for i in range(4):
                    nc.scalar.mul(out=tmp[:, i, :], in_=po[:, (4 + i) * C:(5 + i) * C],
                                  mul=recip[:, 4 + i:5 + i])
                acc = sp.tile([128, C], F32, tag="oacc")
                for h in range(4):
                    nc.vector.scalar_tensor_tensor(
                        out=acc, in0=po[:, h * C:(h + 1) * C],
                        scalar=recip[:, h:h + 1],
                        in1=xr_sb[:, ls, :] if h == 0 else acc,
                        op0=mybir.AluOpType.mult, op1=mybir.AluOpType.add)
                for i in range(4):
                    nc.vector.tensor_tensor(
                        out=acc, in0=acc, in1=tmp[:, i, :],
                        op=mybir.AluOpType.add)
                nc.sync.dma_start(out=OUT[lblk, :], in_=acc)

            for h in range(HEADS):
                emit_head(h, 0)
            for h in range(HEADS):
                emit_head(h, 1)
                if h >= 4:       # spread lh0's Phase D between lh1 heads
                    emit_d(h - 4)
            for ls in range(4, 8):
                emit_d(ls)

    nc.compile()
    return nc


def kernel(x, z, Wk, bk, Wv, bv, Wo, bo):
    x = np.ascontiguousarray(x, dtype=np.float32)
    z = np.ascontiguousarray(z, dtype=np.float32)
    if "nc" not in _CACHE:
        _CACHE["nc"] = build_nc()
    nc = _CACHE["nc"]
    # bv/bo fold: P rows are convex-combination outputs plus bv, so the output
    # picks up the constant bv @ Wo + bo; bk is softmax-shift-invariant.
    bo_eff = (np.asarray(bv, np.float32) @ np.asarray(Wo, np.float32)
              + np.asarray(bo, np.float32))
    shared = {
        "Wk": np.ascontiguousarray(np.asarray(Wk, np.float32).astype(ml_dtypes.bfloat16)),
        "Wv": np.ascontiguousarray(np.asarray(Wv, np.float32).astype(ml_dtypes.bfloat16)),
        "Wo": np.ascontiguousarray(np.asarray(Wo, np.float32)
                                   .reshape(HEADS, C, C).transpose(1, 0, 2)
                                   .astype(ml_dtypes.bfloat16)),
    }
    in_maps = []
    for core in range(N_CORES):
        bi, half = core // 2, core % 2
        xi = x[bi].reshape(C, L)
        xr = (x[bi].reshape(-1)[half * LH * C:(half + 1) * LH * C]
              .reshape(LH // 128, 128, C).transpose(1, 0, 2)
              + bo_eff[None, None, :])
        in_maps.append({
            "xq": np.ascontiguousarray(
                xi[:, half * LH:(half + 1) * LH].astype(ml_dtypes.bfloat16)),
            "xr": np.ascontiguousarray(xr),
            "zb": np.ascontiguousarray(
                z[bi].reshape(DIM, L).astype(ml_dtypes.bfloat16)),
            **shared,
        })
    _CACHE["in_maps"] = in_maps
    res = run_bass_kernel_spmd(nc, in_maps, list(range(N_CORES)))
    full = np.empty((B, L * C), dtype=np.float32)
    for core in range(N_CORES):
        bi, half = core // 2, core % 2
        full[bi, half * LH * C:(half + 1) * LH * C] = \
            res.results[core]["out"].reshape(-1)
    return full.reshape(B, C, H, W)
